# revision 6
# baseline (speedup 1.0000x reference)
"""Trainium2 Bass kernel for a TBN (ternary-binary) ResNet BasicBlock.

    out = x + conv3x3(sign(bn2(conv3x3(sign(bn1(x)), tern(w1)))), tern(w2))

Key facts exploited:
  * binarized activations are exactly {-1,+1} and ternarized weights are
    alpha * {-1,0,+1}; factoring out alpha, both convs reduce to integer
    "count" matmuls whose operands are exactly representable in fp8e4m3.
    PSUM accumulates in fp32, so the conv result is bit-exact.
  * fp8 + MatmulPerfMode.DoubleRow contracts K=256 (both 128-channel
    halves) in a single PE pass at 2 MACs/cell/cycle.
  * eval-mode BN + binarize folds to sign(x*scale + bias) -> one ScalarE
    ACTIVATE(Sign) with per-partition (per-channel) scale/bias APs.
  * 3x3 same-conv over a zero-padded [C, (H+2)*(W+2)] flat image = 9
    shifted-slice matmuls accumulated into PSUM (padding zeros absorb
    all row-wrap artifacts).

Sharding: data-parallel over batch, 8 images per core, weights/BN
replicated (no collectives needed in this forward pass).
"""

import sys

if "/opt/trn_rl_repo" not in sys.path:
    sys.path.insert(0, "/opt/trn_rl_repo")

import numpy as np

import concourse.bass as bass
import concourse.mybir as mybir
from concourse import bacc, tile
from concourse.bass_utils import run_bass_kernel_spmd

B, C, H, W = 64, 256, 32, 32
EPS = 1e-5
N_CORES = 8
PER = B // N_CORES          # images per core
WP = W + 2                  # padded row width (34)
PADIMG = (H + 2) * WP       # 1156 padded pixels per image
PADL = 1184                 # fp8 row allocation; >= 70 + 32*34 = 1158, 16-aligned
HW = H * W                  # 1024
F32 = mybir.dt.float32
FP8 = mybir.dt.float8e4
NP_FP8 = mybir.dt.np(FP8)
ROW_CHUNKS = [(0, 15), (15, 30), (30, 32)]   # <=510 psum cols per chunk

_cache: dict = {}


def _build_program() -> bass.Bass:
    """One SPMD program; all data-dependent scalars come in via `vecs`."""
    nc = bacc.Bacc("TRN2", target_bir_lowering=False, debug=False,
                   num_devices=N_CORES)
    xs = nc.dram_tensor("xs", [PER, C, HW], F32, kind="ExternalInput").ap()
    # wq[k, j, i, m]: j = conv*18 + tap*2 + co_t ; weight = s[co_t*128+m, i*128+k, tap]
    wq = nc.dram_tensor("wq", [128, 36, 2, 128], FP8, kind="ExternalInput").ap()
    # vecs columns: 0,1 inv1(lo,hi) | 2,3 b1 | 4,5 a1*inv2 | 6,7 b2 | 8 alpha2
    vecs = nc.dram_tensor("vecs", [128, 12], F32, kind="ExternalInput").ap()
    out = nc.dram_tensor("out", [PER, C, HW], F32, kind="ExternalOutput").ap()

    sign_f = mybir.ActivationFunctionType.Sign
    dr = mybir.MatmulPerfMode.DoubleRow

    # Wait-slot discipline: TRN2 engine descriptors support only 2 sync
    # waits, so every compute instruction is kept at <=2 distinct
    # cross-engine producers (same-semaphore waits merge):
    #   * BN vectors are copied onto ScalarE once (vec_act) so Sign/Copy
    #     ACTs never wait on the vec DMA.
    #   * both conv drains run on ScalarE, so PE matmuls only ever wait
    #     on {weight DMA, ACT} and psum-slot WARs merge into the ACT wait.
    #   * the residual add writes in-place into the x tile (no out tile).
    with tile.TileContext(nc) as tc:
        with (
            tc.tile_pool(name="wpool", bufs=1) as wpool,
            tc.tile_pool(name="xpool", bufs=6) as xpool,
            tc.tile_pool(name="ppool", bufs=2) as ppool,
            tc.tile_pool(name="tpool", bufs=4) as tpool,
            tc.tile_pool(name="pspool", bufs=8, space="PSUM") as pspool,
        ):
            w_sb = wpool.tile([128, 36, 2, 128], FP8, tag="w")
            nc.sync.dma_start(out=w_sb[:], in_=wq)
            vec_sb = wpool.tile([128, 12], F32, tag="vec")
            nc.sync.dma_start(out=vec_sb[:], in_=vecs)
            vec_act = wpool.tile([128, 12], F32, tag="vec_act")
            nc.scalar.copy(vec_act[:], vec_sb[:])

            def conv(p_in, conv_idx, co_t, r0, r1, psum_tile):
                """accumulate 9 taps of one row-chunk into psum_tile"""
                cols = (r1 - r0) * WP
                g0 = r0 * WP
                for tap in range(9):
                    dy, dx = tap // 3, tap % 3
                    off = dy * WP + dx
                    nc.tensor.matmul(
                        psum_tile[:, :cols],
                        w_sb[:, conv_idx * 18 + tap * 2 + co_t, :, :],
                        p_in[:, :, g0 + off: g0 + off + cols],
                        start=(tap == 0),
                        stop=(tap == 8),
                        perf_mode=dr,
                    )

            for img in range(PER):
                x_sb = []
                for t in range(2):
                    xt = xpool.tile([128, HW], F32, tag="x")
                    nc.sync.dma_start(out=xt[:], in_=xs[img, t * 128:(t + 1) * 128, :])
                    x_sb.append(xt)

                # ---- binarize bn1(x) into padded fp8 image ----
                p1 = ppool.tile([128, 2, PADL], FP8, tag="p1")
                nc.gpsimd.memset(p1[:], 0.0)
                for t in range(2):
                    dst = p1[:, t, WP + 1: WP + 1 + H * WP].rearrange(
                        "p (r c) -> p r c", c=WP)[:, :, 0:W]
                    src = x_sb[t].rearrange("p (r c) -> p r c", c=W)
                    nc.scalar.activation(dst, src, sign_f,
                                         bias=vec_act[:, 2 + t: 3 + t],
                                         scale=vec_act[:, 0 + t: 1 + t])

                # ---- conv1 -> sign(bn2 . alpha1) -> padded fp8 image ----
                p2 = ppool.tile([128, 2, PADL], FP8, tag="p2")
                nc.gpsimd.memset(p2[:], 0.0)
                for co_t in range(2):
                    for (r0, r1) in ROW_CHUNKS:
                        cols = (r1 - r0) * WP
                        ps = pspool.tile([128, 510], F32, tag="ps")
                        conv(p1, 0, co_t, r0, r1, ps)
                        src = ps[:, :cols].rearrange(
                            "p (r c) -> p r c", c=WP)[:, :, 0:W]
                        dst = p2[:, co_t, WP + 1 + r0 * WP: WP + 1 + r1 * WP].rearrange(
                            "p (r c) -> p r c", c=WP)[:, :, 0:W]
                        nc.scalar.activation(dst, src, sign_f,
                                             bias=vec_act[:, 6 + co_t: 7 + co_t],
                                             scale=vec_act[:, 4 + co_t: 5 + co_t])

                # ---- conv2 -> out = x + alpha2 * counts (in-place on x) ----
                for co_t in range(2):
                    for (r0, r1) in ROW_CHUNKS:
                        cols = (r1 - r0) * WP
                        n = (r1 - r0) * W
                        ps = pspool.tile([128, 510], F32, tag="ps")
                        conv(p2, 1, co_t, r0, r1, ps)
                        src = ps[:, :cols].rearrange(
                            "p (r c) -> p r c", c=WP)[:, :, 0:W]
                        tmp = tpool.tile([128, 15 * W], F32, tag="tmp")
                        tview = tmp[:, :n].rearrange("p (r c) -> p r c", c=W)
                        nc.scalar.activation(tview, src,
                                             mybir.ActivationFunctionType.Copy,
                                             scale=vec_act[:, 8:9])
                        xsl = x_sb[co_t][:, r0 * W: r1 * W]
                        nc.vector.tensor_add(out=xsl, in0=xsl, in1=tmp[:, :n])
                    nc.sync.dma_start(
                        out=out[img, co_t * 128:(co_t + 1) * 128, :],
                        in_=x_sb[co_t][:])
    nc.compile()   # bacc pipeline: legalizes >1-wait insts into EventSemaphores
    return nc


def _host_prep(inputs: dict) -> tuple:
    """Fold BN params, ternarize weights, pack fp8 weight tensor."""
    def fold(g, b, m, v):
        inv = (g / np.sqrt(v + EPS)).astype(np.float32)
        return inv, (b - m * inv).astype(np.float32)

    inv1, b1 = fold(inputs["bn1_gamma"], inputs["bn1_beta"],
                    inputs["bn1_mean"], inputs["bn1_var"])
    inv2, b2 = fold(inputs["bn2_gamma"], inputs["bn2_beta"],
                    inputs["bn2_mean"], inputs["bn2_var"])

    def tern(w):
        aw = np.abs(w)
        delta = np.float32(0.7) * aw.mean(dtype=np.float32)
        mask = aw > delta
        alpha = np.float32((aw * mask).sum(dtype=np.float32) / max(mask.sum(), 1.0))
        return alpha, (np.sign(w) * mask).astype(np.float32)

    a1, s1 = tern(inputs["w1"])
    a2, s2 = tern(inputs["w2"])

    # pack wq[k, conv*18 + tap*2 + co_t, i, m] = s[co_t*128+m, i*128+k, dy, dx]
    def pack(s):
        a = s.reshape(2, 128, 2, 128, 3, 3)           # [co_t, m, i, k, dy, dx]
        a = np.transpose(a, (3, 4, 5, 0, 2, 1))       # [k, dy, dx, co_t, i, m]
        return a.reshape(128, 18, 2, 128)

    wq = np.concatenate([pack(s1), pack(s2)], axis=1).astype(NP_FP8)

    vecs = np.zeros((128, 12), np.float32)
    vecs[:, 0] = inv1[:128]
    vecs[:, 1] = inv1[128:]
    vecs[:, 2] = b1[:128]
    vecs[:, 3] = b1[128:]
    vecs[:, 4] = (a1 * inv2)[:128]
    vecs[:, 5] = (a1 * inv2)[128:]
    vecs[:, 6] = b2[:128]
    vecs[:, 7] = b2[128:]
    vecs[:, 8] = a2
    return wq, vecs


def _get_program() -> bass.Bass:
    if "nc" not in _cache:
        _cache["nc"] = _build_program()
    return _cache["nc"]


def make_in_maps(inputs: dict) -> list:
    inputs = {k: np.asarray(v) for k, v in inputs.items()}
    wq, vecs = _host_prep(inputs)
    x = np.ascontiguousarray(inputs["x"].astype(np.float32).reshape(B, C, HW))
    in_maps = []
    for c in range(N_CORES):
        in_maps.append({
            "xs": np.ascontiguousarray(x[c * PER:(c + 1) * PER]),
            "wq": wq,
            "vecs": vecs,
        })
    return in_maps


def run(inputs: dict, trace: bool = False):
    nc = _get_program()
    in_maps = make_in_maps(inputs)
    res = run_bass_kernel_spmd(nc, in_maps, list(range(N_CORES)), trace=trace)
    out = np.concatenate(
        [res.results[c]["out"].reshape(PER, C, H, W) for c in range(N_CORES)],
        axis=0).astype(np.float32)
    return out, res


def kernel(**inputs) -> np.ndarray:
    out, _ = run(inputs)
    return out


# revision 15
# speedup vs baseline: 1.1150x; 1.1150x over previous
"""Trainium2 Bass kernel for a TBN (ternary-binary) ResNet BasicBlock.

    out = x + conv3x3(sign(bn2(conv3x3(sign(bn1(x)), tern(w1)))), tern(w2))

Key facts exploited:
  * binarized activations are exactly {-1,+1} and ternarized weights are
    alpha * {-1,0,+1}; factoring out alpha, both convs reduce to integer
    "count" matmuls whose operands are exactly representable in fp8e4m3.
    PSUM accumulates in fp32, so the conv result is bit-exact.
  * fp8 + MatmulPerfMode.DoubleRow contracts K=256 (both 128-channel
    halves) in a single PE pass at 2 MACs/cell/cycle.
  * eval-mode BN + binarize folds to sign(x*scale + bias) -> one ScalarE
    ACTIVATE(Sign) with per-partition (per-channel) scale/bias APs.
  * 3x3 same-conv over a zero-padded [C, (H+2)*(W+2)] flat image = 9
    shifted-slice matmuls accumulated into PSUM (padding zeros absorb
    all row-wrap artifacts).

Sharding: data-parallel over batch, 8 images per core, weights/BN
replicated (no collectives needed in this forward pass).
"""

import sys

if "/opt/trn_rl_repo" not in sys.path:
    sys.path.insert(0, "/opt/trn_rl_repo")

import numpy as np

import concourse.bass as bass
import concourse.mybir as mybir
from concourse import bacc, tile
from concourse.bass_utils import run_bass_kernel_spmd

B, C, H, W = 64, 256, 32, 32
EPS = 1e-5
N_CORES = 8
PER = B // N_CORES          # images per core
WP = W + 2                  # padded row width (34)
PADIMG = (H + 2) * WP       # 1156 padded pixels per image
PADL = 1184                 # fp8 row allocation; >= 70 + 32*34 = 1158, 16-aligned
HW = H * W                  # 1024
F32 = mybir.dt.float32
FP8 = mybir.dt.float8e4
NP_FP8 = mybir.dt.np(FP8)
ROW_CHUNKS = [(0, 16), (16, 32)]   # 16 rows x 32 valid cols = 512 psum cols
VECB = 48                          # vecs bytes at head of the packed wq tensor
WQB = VECB + 36 * 256              # 9264 bytes per partition

_cache: dict = {}


def _build_program() -> bass.Bass:
    """One SPMD program; all data-dependent scalars come in via `vecs`."""
    nc = bacc.Bacc("TRN2", target_bir_lowering=False, debug=False,
                   num_devices=N_CORES)
    xs = nc.dram_tensor("xs", [PER, C, HW], F32, kind="ExternalInput").ap()
    # Packed constants, one contiguous DMA-friendly tensor per partition row:
    #   bytes [0:48)    = vecs[12] f32: 0,1 inv1(lo,hi) | 2,3 b1 | 4,5 a1*inv2
    #                     | 6,7 b2 | 8 alpha2
    #   bytes [48:9264) = fp8 weights wq[j, i, m], j = conv*18 + tap*2 + co_t;
    #                     weight = s[co_t*128+m, i*128+k, tap] on partition k
    wq = nc.dram_tensor("wq", [128, WQB], mybir.dt.uint8, kind="ExternalInput").ap()
    out = nc.dram_tensor("out", [PER, C, HW], F32, kind="ExternalOutput").ap()

    sign_f = mybir.ActivationFunctionType.Sign
    dr = mybir.MatmulPerfMode.DoubleRow

    # Wait-slot discipline: TRN2 engine descriptors support only 2 sync
    # waits, so every compute instruction is kept at <=2 distinct
    # cross-engine producers (same-semaphore waits merge):
    #   * BN vectors are copied onto ScalarE once (vec_act) so Sign/Copy
    #     ACTs never wait on the vec DMA.
    #   * both conv drains run on ScalarE, so PE matmuls only ever wait
    #     on {weight DMA, ACT} and psum-slot WARs merge into the ACT wait.
    #   * the residual add writes in-place into the x tile (no out tile).
    with tile.TileContext(nc) as tc:
        with (
            tc.tile_pool(name="wpool", bufs=1) as wpool,
            tc.tile_pool(name="xpool", bufs=6) as xpool,
            tc.tile_pool(name="ppool", bufs=2) as ppool,
            tc.tile_pool(name="tpool", bufs=4) as tpool,
            tc.tile_pool(name="pspool", bufs=8, space="PSUM") as pspool,
        ):
            w_sb = wpool.tile([128, WQB], mybir.dt.uint8, tag="w")
            half = VECB + 18 * 256   # vecs + conv1 weights
            nc.sync.dma_start(out=w_sb[:, :half], in_=wq[:, :half])
            nc.sync.dma_start(out=w_sb[:, half:], in_=wq[:, half:])
            wview = w_sb[:, VECB:].bitcast(FP8).rearrange("p (j i m) -> p j i m", i=2, m=128)
            vec_sb = w_sb[:, :VECB].bitcast(F32)
            vec_act = wpool.tile([128, 12], F32, tag="vec_act")
            nc.scalar.copy(vec_act[:], vec_sb)

            def conv(p_in, conv_idx, co_t, r0, r1, psum_tile):
                """accumulate 9 taps of one row-chunk into psum_tile.

                The rhs is a 4D AP [K, 2, rows, 32-of-34] that skips the
                two pad columns per image row, so only valid output pixels
                are streamed through the PE."""
                rows = r1 - r0
                n = rows * W
                for tap in range(9):
                    dy, dx = tap // 3, tap % 3
                    start = (r0 + dy) * WP + dx
                    rhs = p_in[:, :, start: start + rows * WP].rearrange(
                        "p i (r c) -> p i r c", c=WP)[:, :, :, 0:W]
                    nc.tensor.matmul(
                        psum_tile[:, :n],
                        wview[:, conv_idx * 18 + tap * 2 + co_t, :, :],
                        rhs,
                        start=(tap == 0),
                        stop=(tap == 8),
                        perf_mode=dr,
                    )

            for img in range(PER):
                x_sb = []
                for t in range(2):
                    xt = xpool.tile([128, HW], F32, tag="x")
                    nc.sync.dma_start(out=xt[:], in_=xs[img, t * 128:(t + 1) * 128, :])
                    x_sb.append(xt)

                # ---- binarize bn1(x) into padded fp8 image ----
                p1 = ppool.tile([128, 2, PADL], FP8, tag="p1")
                nc.gpsimd.memset(p1[:], 0.0)
                for t in range(2):
                    dst = p1[:, t, WP + 1: WP + 1 + H * WP].rearrange(
                        "p (r c) -> p r c", c=WP)[:, :, 0:W]
                    src = x_sb[t].rearrange("p (r c) -> p r c", c=W)
                    nc.scalar.activation(dst, src, sign_f,
                                         bias=vec_act[:, 2 + t: 3 + t],
                                         scale=vec_act[:, 0 + t: 1 + t])

                # ---- conv1 -> sign(bn2 . alpha1) -> padded fp8 image ----
                p2 = ppool.tile([128, 2, PADL], FP8, tag="p2")
                nc.gpsimd.memset(p2[:], 0.0)
                for co_t in range(2):
                    for (r0, r1) in ROW_CHUNKS:
                        n = (r1 - r0) * W
                        ps = pspool.tile([128, 512], F32, tag="ps")
                        conv(p1, 0, co_t, r0, r1, ps)
                        src = ps[:, :n].rearrange("p (r c) -> p r c", c=W)
                        dst = p2[:, co_t, WP + 1 + r0 * WP: WP + 1 + r1 * WP].rearrange(
                            "p (r c) -> p r c", c=WP)[:, :, 0:W]
                        nc.scalar.activation(dst, src, sign_f,
                                             bias=vec_act[:, 6 + co_t: 7 + co_t],
                                             scale=vec_act[:, 4 + co_t: 5 + co_t])

                # ---- conv2 -> out = x + alpha2 * counts (in-place on x) ----
                for co_t in range(2):
                    for (r0, r1) in ROW_CHUNKS:
                        n = (r1 - r0) * W
                        ps = pspool.tile([128, 512], F32, tag="ps")
                        conv(p2, 1, co_t, r0, r1, ps)
                        tmp = tpool.tile([128, 512], F32, tag="tmp")
                        nc.scalar.activation(tmp[:, :n], ps[:, :n],
                                             mybir.ActivationFunctionType.Copy,
                                             scale=vec_act[:, 8:9])
                        xsl = x_sb[co_t][:, r0 * W: r1 * W]
                        nc.vector.tensor_add(out=xsl, in0=xsl, in1=tmp[:, :n])
                    nc.sync.dma_start(
                        out=out[img, co_t * 128:(co_t + 1) * 128, :],
                        in_=x_sb[co_t][:])
    nc.compile()   # bacc pipeline: legalizes >1-wait insts into EventSemaphores
    return nc


def _host_prep(inputs: dict) -> tuple:
    """Fold BN params, ternarize weights, pack fp8 weight tensor."""
    def fold(g, b, m, v):
        inv = (g / np.sqrt(v + EPS)).astype(np.float32)
        return inv, (b - m * inv).astype(np.float32)

    inv1, b1 = fold(inputs["bn1_gamma"], inputs["bn1_beta"],
                    inputs["bn1_mean"], inputs["bn1_var"])
    inv2, b2 = fold(inputs["bn2_gamma"], inputs["bn2_beta"],
                    inputs["bn2_mean"], inputs["bn2_var"])

    def tern(w):
        aw = np.abs(w)
        delta = np.float32(0.7) * aw.mean(dtype=np.float32)
        mask = aw > delta
        alpha = np.float32((aw * mask).sum(dtype=np.float32) / max(mask.sum(), 1.0))
        return alpha, (np.sign(w) * mask).astype(np.float32)

    a1, s1 = tern(inputs["w1"])
    a2, s2 = tern(inputs["w2"])

    # pack wq[k, conv*18 + tap*2 + co_t, i, m] = s[co_t*128+m, i*128+k, dy, dx]
    def pack(s):
        a = s.reshape(2, 128, 2, 128, 3, 3)           # [co_t, m, i, k, dy, dx]
        a = np.transpose(a, (3, 4, 5, 0, 2, 1))       # [k, dy, dx, co_t, i, m]
        return a.reshape(128, 18 * 2 * 128)

    vecs = np.zeros((128, 12), np.float32)
    vecs[:, 0] = inv1[:128]
    vecs[:, 1] = inv1[128:]
    vecs[:, 2] = b1[:128]
    vecs[:, 3] = b1[128:]
    vecs[:, 4] = (a1 * inv2)[:128]
    vecs[:, 5] = (a1 * inv2)[128:]
    vecs[:, 6] = b2[:128]
    vecs[:, 7] = b2[128:]
    vecs[:, 8] = a2

    wq = np.empty((128, WQB), np.uint8)
    wq[:, :VECB] = vecs.view(np.uint8)
    wq[:, VECB:] = np.ascontiguousarray(
        np.concatenate([pack(s1), pack(s2)], axis=1).astype(NP_FP8)).view(np.uint8)
    return wq


def _get_program() -> bass.Bass:
    if "nc" not in _cache:
        _cache["nc"] = _build_program()
    return _cache["nc"]


def make_in_maps(inputs: dict) -> list:
    inputs = {k: np.asarray(v) for k, v in inputs.items()}
    wq = _host_prep(inputs)
    x = np.ascontiguousarray(inputs["x"].astype(np.float32).reshape(B, C, HW))
    in_maps = []
    for c in range(N_CORES):
        in_maps.append({
            "xs": np.ascontiguousarray(x[c * PER:(c + 1) * PER]),
            "wq": wq,
        })
    return in_maps


def run(inputs: dict, trace: bool = False):
    nc = _get_program()
    in_maps = make_in_maps(inputs)
    res = run_bass_kernel_spmd(nc, in_maps, list(range(N_CORES)), trace=trace)
    out = np.concatenate(
        [res.results[c]["out"].reshape(PER, C, H, W) for c in range(N_CORES)],
        axis=0).astype(np.float32)
    return out, res


def kernel(**inputs) -> np.ndarray:
    out, _ = run(inputs)
    return out


# revision 17
# speedup vs baseline: 1.1258x; 1.0097x over previous
"""Trainium2 Bass kernel for a TBN (ternary-binary) ResNet BasicBlock.

    out = x + conv3x3(sign(bn2(conv3x3(sign(bn1(x)), tern(w1)))), tern(w2))

Key facts exploited:
  * binarized activations are exactly {-1,+1} and ternarized weights are
    alpha * {-1,0,+1}; factoring out alpha, both convs reduce to integer
    "count" matmuls whose operands are exactly representable in fp8e4m3.
    PSUM accumulates in fp32, so the conv result is bit-exact.
  * fp8 + MatmulPerfMode.DoubleRow contracts K=256 (both 128-channel
    halves) in a single PE pass at 2 MACs/cell/cycle.
  * eval-mode BN + binarize folds to sign(x*scale + bias) -> one ScalarE
    ACTIVATE(Sign) with per-partition (per-channel) scale/bias APs.
  * 3x3 same-conv over a zero-padded [C, (H+2)*(W+2)] flat image = 9
    shifted-slice matmuls accumulated into PSUM (padding zeros absorb
    all row-wrap artifacts).

Sharding: data-parallel over batch, 8 images per core, weights/BN
replicated (no collectives needed in this forward pass).
"""

import sys

if "/opt/trn_rl_repo" not in sys.path:
    sys.path.insert(0, "/opt/trn_rl_repo")

import numpy as np

import concourse.bass as bass
import concourse.mybir as mybir
from concourse import bacc, tile
from concourse.bass_utils import run_bass_kernel_spmd

B, C, H, W = 64, 256, 32, 32
EPS = 1e-5
N_CORES = 8
PER = B // N_CORES          # images per core
WP = W + 2                  # padded row width (34)
PADIMG = (H + 2) * WP       # 1156 padded pixels per image
PADL = 1184                 # fp8 row allocation; >= 70 + 32*34 = 1158, 16-aligned
HW = H * W                  # 1024
F32 = mybir.dt.float32
FP8 = mybir.dt.float8e4
NP_FP8 = mybir.dt.np(FP8)
ROW_CHUNKS = [(0, 16), (16, 32)]   # 16 rows x 32 valid cols = 512 psum cols
VECB = 48                          # vecs bytes at head of the packed wq tensor
WQB = VECB + 36 * 256              # 9264 bytes per partition

_cache: dict = {}


def _build_program() -> bass.Bass:
    """One SPMD program; all data-dependent scalars come in via `vecs`."""
    nc = bacc.Bacc("TRN2", target_bir_lowering=False, debug=False,
                   num_devices=N_CORES)
    xs = nc.dram_tensor("xs", [PER, C, HW], F32, kind="ExternalInput").ap()
    # Packed constants, one contiguous DMA-friendly tensor per partition row:
    #   bytes [0:48)    = vecs[12] f32: 0,1 inv1(lo,hi) | 2,3 b1 | 4,5 a1*inv2
    #                     | 6,7 b2 | 8 alpha2
    #   bytes [48:9264) = fp8 weights wq[j, i, m], j = conv*18 + tap*2 + co_t;
    #                     weight = s[co_t*128+m, i*128+k, tap] on partition k
    wq = nc.dram_tensor("wq", [128, WQB], mybir.dt.uint8, kind="ExternalInput").ap()
    out = nc.dram_tensor("out", [PER, C, HW], F32, kind="ExternalOutput").ap()

    sign_f = mybir.ActivationFunctionType.Sign
    dr = mybir.MatmulPerfMode.DoubleRow

    # Wait-slot discipline: TRN2 engine descriptors support only 2 sync
    # waits, so every compute instruction is kept at <=2 distinct
    # cross-engine producers (same-semaphore waits merge):
    #   * BN vectors are copied onto ScalarE once (vec_act) so Sign/Copy
    #     ACTs never wait on the vec DMA.
    #   * both conv drains run on ScalarE, so PE matmuls only ever wait
    #     on {weight DMA, ACT} and psum-slot WARs merge into the ACT wait.
    #   * the residual add writes in-place into the x tile (no out tile).
    with tile.TileContext(nc) as tc:
        with (
            tc.tile_pool(name="wpool", bufs=1) as wpool,
            tc.tile_pool(name="xpool", bufs=6) as xpool,
            tc.tile_pool(name="ppool", bufs=2) as ppool,
            tc.tile_pool(name="tpool", bufs=4) as tpool,
            tc.tile_pool(name="pspool", bufs=8, space="PSUM") as pspool,
        ):
            w_sb = wpool.tile([128, WQB], mybir.dt.uint8, tag="w")
            half = VECB + 18 * 256   # vecs + conv1 weights
            nc.sync.dma_start(out=w_sb[:, :half], in_=wq[:, :half])
            wview = w_sb[:, VECB:].bitcast(FP8).rearrange("p (j i m) -> p j i m", i=2, m=128)
            vec_sb = w_sb[:, :VECB].bitcast(F32)
            vec_act = wpool.tile([128, 12], F32, tag="vec_act")
            nc.scalar.copy(vec_act[:], vec_sb)

            # img0's x tiles gate the first Sign -> first matmul; issue them
            # on the serial DMA queue before the conv2 weight half.
            x0_pre = []
            for t in range(2):
                xt = xpool.tile([128, HW], F32, tag="x")
                nc.sync.dma_start(out=xt[:], in_=xs[0, t * 128:(t + 1) * 128, :])
                x0_pre.append(xt)
            nc.sync.dma_start(out=w_sb[:, half:], in_=wq[:, half:])

            def conv(p_in, conv_idx, co_t, r0, r1, psum_tile):
                """accumulate 9 taps of one row-chunk into psum_tile.

                The rhs is a 4D AP [K, 2, rows, 32-of-34] that skips the
                two pad columns per image row, so only valid output pixels
                are streamed through the PE."""
                rows = r1 - r0
                n = rows * W
                for tap in range(9):
                    dy, dx = tap // 3, tap % 3
                    start = (r0 + dy) * WP + dx
                    rhs = p_in[:, :, start: start + rows * WP].rearrange(
                        "p i (r c) -> p i r c", c=WP)[:, :, :, 0:W]
                    nc.tensor.matmul(
                        psum_tile[:, :n],
                        wview[:, conv_idx * 18 + tap * 2 + co_t, :, :],
                        rhs,
                        start=(tap == 0),
                        stop=(tap == 8),
                        perf_mode=dr,
                    )

            for img in range(PER):
                if img == 0:
                    x_sb = x0_pre
                else:
                    x_sb = []
                    for t in range(2):
                        xt = xpool.tile([128, HW], F32, tag="x")
                        nc.sync.dma_start(
                            out=xt[:], in_=xs[img, t * 128:(t + 1) * 128, :])
                        x_sb.append(xt)

                # ---- binarize bn1(x) into padded fp8 image ----
                p1 = ppool.tile([128, 2, PADL], FP8, tag="p1")
                nc.gpsimd.memset(p1[:], 0.0)
                for t in range(2):
                    dst = p1[:, t, WP + 1: WP + 1 + H * WP].rearrange(
                        "p (r c) -> p r c", c=WP)[:, :, 0:W]
                    src = x_sb[t].rearrange("p (r c) -> p r c", c=W)
                    nc.scalar.activation(dst, src, sign_f,
                                         bias=vec_act[:, 2 + t: 3 + t],
                                         scale=vec_act[:, 0 + t: 1 + t])

                # ---- conv1 -> sign(bn2 . alpha1) -> padded fp8 image ----
                p2 = ppool.tile([128, 2, PADL], FP8, tag="p2")
                nc.gpsimd.memset(p2[:], 0.0)
                for co_t in range(2):
                    for (r0, r1) in ROW_CHUNKS:
                        n = (r1 - r0) * W
                        ps = pspool.tile([128, 512], F32, tag="ps")
                        conv(p1, 0, co_t, r0, r1, ps)
                        src = ps[:, :n].rearrange("p (r c) -> p r c", c=W)
                        dst = p2[:, co_t, WP + 1 + r0 * WP: WP + 1 + r1 * WP].rearrange(
                            "p (r c) -> p r c", c=WP)[:, :, 0:W]
                        nc.scalar.activation(dst, src, sign_f,
                                             bias=vec_act[:, 6 + co_t: 7 + co_t],
                                             scale=vec_act[:, 4 + co_t: 5 + co_t])

                # ---- conv2 -> out = x + alpha2 * counts (in-place on x) ----
                for co_t in range(2):
                    for (r0, r1) in ROW_CHUNKS:
                        n = (r1 - r0) * W
                        ps = pspool.tile([128, 512], F32, tag="ps")
                        conv(p2, 1, co_t, r0, r1, ps)
                        tmp = tpool.tile([128, 512], F32, tag="tmp")
                        nc.scalar.activation(tmp[:, :n], ps[:, :n],
                                             mybir.ActivationFunctionType.Copy,
                                             scale=vec_act[:, 8:9])
                        xsl = x_sb[co_t][:, r0 * W: r1 * W]
                        nc.vector.tensor_add(out=xsl, in0=xsl, in1=tmp[:, :n])
                    nc.sync.dma_start(
                        out=out[img, co_t * 128:(co_t + 1) * 128, :],
                        in_=x_sb[co_t][:])
    nc.compile()   # bacc pipeline: legalizes >1-wait insts into EventSemaphores
    return nc


def _host_prep(inputs: dict) -> tuple:
    """Fold BN params, ternarize weights, pack fp8 weight tensor."""
    def fold(g, b, m, v):
        inv = (g / np.sqrt(v + EPS)).astype(np.float32)
        return inv, (b - m * inv).astype(np.float32)

    inv1, b1 = fold(inputs["bn1_gamma"], inputs["bn1_beta"],
                    inputs["bn1_mean"], inputs["bn1_var"])
    inv2, b2 = fold(inputs["bn2_gamma"], inputs["bn2_beta"],
                    inputs["bn2_mean"], inputs["bn2_var"])

    def tern(w):
        aw = np.abs(w)
        delta = np.float32(0.7) * aw.mean(dtype=np.float32)
        mask = aw > delta
        alpha = np.float32((aw * mask).sum(dtype=np.float32) / max(mask.sum(), 1.0))
        return alpha, (np.sign(w) * mask).astype(np.float32)

    a1, s1 = tern(inputs["w1"])
    a2, s2 = tern(inputs["w2"])

    # pack wq[k, conv*18 + tap*2 + co_t, i, m] = s[co_t*128+m, i*128+k, dy, dx]
    def pack(s):
        a = s.reshape(2, 128, 2, 128, 3, 3)           # [co_t, m, i, k, dy, dx]
        a = np.transpose(a, (3, 4, 5, 0, 2, 1))       # [k, dy, dx, co_t, i, m]
        return a.reshape(128, 18 * 2 * 128)

    vecs = np.zeros((128, 12), np.float32)
    vecs[:, 0] = inv1[:128]
    vecs[:, 1] = inv1[128:]
    vecs[:, 2] = b1[:128]
    vecs[:, 3] = b1[128:]
    vecs[:, 4] = (a1 * inv2)[:128]
    vecs[:, 5] = (a1 * inv2)[128:]
    vecs[:, 6] = b2[:128]
    vecs[:, 7] = b2[128:]
    vecs[:, 8] = a2

    wq = np.empty((128, WQB), np.uint8)
    wq[:, :VECB] = vecs.view(np.uint8)
    wq[:, VECB:] = np.ascontiguousarray(
        np.concatenate([pack(s1), pack(s2)], axis=1).astype(NP_FP8)).view(np.uint8)
    return wq


def _get_program() -> bass.Bass:
    if "nc" not in _cache:
        _cache["nc"] = _build_program()
    return _cache["nc"]


def make_in_maps(inputs: dict) -> list:
    inputs = {k: np.asarray(v) for k, v in inputs.items()}
    wq = _host_prep(inputs)
    x = np.ascontiguousarray(inputs["x"].astype(np.float32).reshape(B, C, HW))
    in_maps = []
    for c in range(N_CORES):
        in_maps.append({
            "xs": np.ascontiguousarray(x[c * PER:(c + 1) * PER]),
            "wq": wq,
        })
    return in_maps


def run(inputs: dict, trace: bool = False):
    nc = _get_program()
    in_maps = make_in_maps(inputs)
    res = run_bass_kernel_spmd(nc, in_maps, list(range(N_CORES)), trace=trace)
    out = np.concatenate(
        [res.results[c]["out"].reshape(PER, C, H, W) for c in range(N_CORES)],
        axis=0).astype(np.float32)
    return out, res


def kernel(**inputs) -> np.ndarray:
    out, _ = run(inputs)
    return out
